# revision 3
# baseline (speedup 1.0000x reference)
"""Distributed causal self-attention for 8 Trainium2 NeuronCores.

Problem: x[2,2048,1024] @ w_qkv[1024,3072] -> causal MHA (16 heads, d=64)
         -> @ w_out[1024,1024]. All fp32.

Sharding: core c (0..7) handles batch b=c//4 and head group g=c%4 (4 heads).
Each core projects qkv for its heads, runs flash attention (transposed-score
layout), then an AllToAll within each 4-core batch group converts head-
parallel attention output into token-parallel slices for the output
projection.  Core c writes output rows [b, 512*g : 512*(g+1), :].

Matmuls run in float32r (TF32-like, full PE rate); softmax in fp32.
"""

import sys

for _p in ("/opt/trn_rl_repo", "/root/.axon_site/_ro/trn_rl_repo"):
    if _p not in sys.path:
        sys.path.insert(0, _p)

import numpy as np

import concourse.bass as bass  # noqa: F401  (bass types used via tile/bacc)
import concourse.mybir as mybir
import concourse.tile as tile
from concourse import bacc
from concourse.bass_utils import run_bass_kernel_spmd

P = 128
B, T, C = 2, 2048, 1024
H, D = 16, 64
HL = 4               # heads per core
DL = HL * D          # 256 local head dims
KC = C // P          # 8 contraction tiles over C
QB = 512             # query chunk
NQ = T // QB         # 4 query chunks
NT = T // P          # 16 token tiles
G = 4                # cores per batch group
TS = T // G          # 512-token output slice per core
SCALE = 1.0 / 8.0    # 1/sqrt(64)
NEG = -1.0e30

F32 = mybir.dt.float32
F32R = mybir.dt.float32r

_CACHED = {}


def _mask_data():
    # tril mask: 0 where key j <= query i, NEG above the diagonal
    j = np.arange(P)[:, None]
    i = np.arange(P)[None, :]
    return np.where(j <= i, 0.0, NEG).astype(np.float32)


def _build():
    nc = bacc.Bacc("TRN2", target_bir_lowering=False, debug=False,
                   num_devices=8)

    xT = nc.dram_tensor("xT", [C, T], F32R, kind="ExternalInput")
    wq = nc.dram_tensor("wq", [C, DL], F32R, kind="ExternalInput")
    wk = nc.dram_tensor("wk", [C, DL], F32R, kind="ExternalInput")
    wv = nc.dram_tensor("wv", [C, DL], F32R, kind="ExternalInput")
    bq = nc.dram_tensor("bq", [1, DL], F32R, kind="ExternalInput")
    bk = nc.dram_tensor("bk", [1, DL], F32R, kind="ExternalInput")
    bv = nc.dram_tensor("bv", [1, DL], F32R, kind="ExternalInput")
    wo = nc.dram_tensor("wo", [DL, C], F32R, kind="ExternalInput")
    bo = nc.dram_tensor("bo", [1, C], F32R, kind="ExternalInput")
    # per query-chunk ReduceScatter slices: rows qc*512 + g*128 .. +128
    out = nc.dram_tensor("out", [NQ, P, C], F32, kind="ExternalOutput")

    masks_dram = nc.inline_tensor(_mask_data(), name="cmasks")

    with tile.TileContext(nc) as tc:
        with (
            tc.tile_pool(name="const", bufs=1) as cp,
            tc.tile_pool(name="persist", bufs=1) as pp,
            tc.tile_pool(name="work", bufs=3) as wk_p,
            tc.tile_pool(name="dram", bufs=1, space="DRAM") as dp,
            tc.tile_pool(name="ps_proj", bufs=2, space="PSUM") as ps_proj,
            tc.tile_pool(name="ps_sT", bufs=2, space="PSUM") as ps_sT,
            tc.tile_pool(name="ps_pv", bufs=2, space="PSUM") as ps_pv,
        ):
            # ---- constants ----
            masks = cp.tile([P, P], F32)
            nc.sync.dma_start(masks[:], masks_dram[:])
            ones_f = cp.tile([1, QB], F32)
            nc.vector.memset(ones_f[:], 1.0)
            ones_r = cp.tile([1, QB], F32R)
            nc.vector.tensor_copy(ones_r[:], ones_f[:])
            # q/k biases as per-partition columns [128, 2] (mi-major)
            bq_col = cp.tile([P, 2], F32)
            bk_col = cp.tile([P, 2], F32)
            nc.sync.dma_start(
                bq_col[:], bq.bitcast(F32)[0, :].rearrange("(m p) -> p m", p=P))
            nc.sync.dma_start(
                bk_col[:], bk.bitcast(F32)[0, :].rearrange("(m p) -> p m", p=P))
            bv_sb = cp.tile([1, DL], F32R)
            bo_sb = cp.tile([1, C], F32R)
            nc.sync.dma_start(bv_sb[:], bv[:])
            nc.sync.dma_start(bo_sb[:], bo[:])

            # ---- persistent activations ----
            qT_sb = pp.tile([P, 2, T], F32R)     # [d, t], d = mi*128+p
            kT_sb = pp.tile([P, 2, T], F32R)
            v_sb = pp.tile([P, NT, HL * (D + 1)], F32R)  # per head: 64 v + ones
            aoT_sb = pp.tile([P, 2, T], F32R)    # attention out^T (normalized)

            # ones columns of v_sb (softmax denominator accumulator)
            ones64 = cp.tile([P, NT * HL], F32)
            nc.vector.memset(ones64[:], 1.0)
            vones = v_sb.rearrange("p n (h e) -> p n h e", h=HL)[:, :, :, D:D + 1]
            nc.vector.tensor_copy(vones, ones64[:].rearrange(
                "p (n h) -> p n h", n=NT)[:, :, :, None])

            with tc.tile_pool(name="xw", bufs=1) as xw:
                xTr = xw.tile([P, KC, T], F32R)
                for kk in range(KC):
                    nc.sync.dma_start(
                        xTr[:, kk, :],
                        xT.rearrange("(k p) t -> k p t", p=P)[kk])
                wq_sb = xw.tile([P, KC, DL], F32R)
                wk_sb = xw.tile([P, KC, DL], F32R)
                wv_sb = xw.tile([P, KC, DL], F32R)
                nc.sync.dma_start(wq_sb[:], wq.rearrange("(k p) m -> p k m", p=P))
                nc.sync.dma_start(wk_sb[:], wk.rearrange("(k p) m -> p k m", p=P))
                nc.sync.dma_start(wv_sb[:], wv.rearrange("(k p) m -> p k m", p=P))

                # ---- phase A: qkv projection ----
                for w_sb, b_col, dst in ((wq_sb, bq_col, qT_sb),
                                         (wk_sb, bk_col, kT_sb)):
                    for mi in range(2):
                        for ni in range(NQ):
                            ps = ps_proj.tile([P, QB], F32, name="proj_ps",
                                              tag="proj_ps")
                            for kk in range(KC):
                                nc.tensor.matmul(
                                    ps[:],
                                    w_sb[:, kk, mi * P:(mi + 1) * P],
                                    xTr[:, kk, ni * QB:(ni + 1) * QB],
                                    start=(kk == 0), stop=(kk == KC - 1))
                            nc.vector.tensor_scalar_add(
                                dst[:, mi, ni * QB:(ni + 1) * QB], ps[:],
                                b_col[:, mi:mi + 1])
                for ti in range(NT):
                    ps = ps_proj.tile([P, DL], F32, name="proj_ps",
                                      tag="proj_ps")
                    for kk in range(KC):
                        nc.tensor.matmul(ps[:], xTr[:, kk, ti * P:(ti + 1) * P],
                                         wv_sb[:, kk, :],
                                         start=(kk == 0), stop=False)
                    nc.tensor.matmul(ps[:], ones_r[:, :P], bv_sb[:],
                                     start=False, stop=True)
                    nc.vector.tensor_copy(
                        v_sb.rearrange("p n (h e) -> p n h e", h=HL)
                        [:, ti, :, 0:D],
                        ps[:].rearrange("p (h e) -> p h e", e=D))
            # xw pool released; wo loads overlap attention below.

            with tc.tile_pool(name="wo_pool", bufs=1) as wop:
                wo_sb = wop.tile([P, 2, C], F32R)
                nc.sync.dma_start(wo_sb[:],
                                  wo.rearrange("(k p) n -> p k n", p=P))
                bo_bc = wop.tile([P, C], F32)
                nc.gpsimd.partition_broadcast(bo_bc[:], bo_sb[:].bitcast(F32))

                # ---- phases B+C interleaved per query chunk ----
                BF16 = mybir.dt.bfloat16
                part_dram = dp.tile([T, C], BF16)
                rs_out = dp.tile([NQ, P, C], BF16)

                def outproj_jobs(qc):
                    # 8 projection psum-groups + deferred RS for chunk qc;
                    # emitted one at a time inside the NEXT chunk's attention
                    # stream as exp-independent PE gap filler.
                    jobs = []

                    def group(mi2, ni):
                        ps = ps_proj.tile([P, QB], F32, name="proj_ps",
                                          tag="proj_ps")
                        for kk in range(2):
                            nc.tensor.matmul(
                                ps[:],
                                aoT_sb[:, kk, mi2 * P:(mi2 + 1) * P],
                                wo_sb[:, kk, ni * QB:(ni + 1) * QB],
                                start=(kk == 0), stop=(kk == 1))
                        o_sb = wk_p.tile([P, QB], BF16, name="o_sb",
                                         tag="o_sb", bufs=2)
                        nc.scalar.copy(o_sb[:], ps[:])
                        nc.sync.dma_start(
                            part_dram[mi2 * P:(mi2 + 1) * P,
                                      ni * QB:(ni + 1) * QB],
                            o_sb[:])

                    for mi2 in range(4 * qc, 4 * qc + 4):
                        for ni in range(2):
                            jobs.append(lambda mi2=mi2, ni=ni:
                                        group(mi2, ni))

                    def rs_job():
                        nc.gpsimd.collective_compute(
                            "ReduceScatter",
                            mybir.AluOpType.add,
                            replica_groups=[[0, 1, 2, 3], [4, 5, 6, 7]],
                            ins=[part_dram[qc * QB:(qc + 1) * QB, :]],
                            outs=[rs_out[qc]],
                        )
                        r_sb = wk_p.tile([P, C], BF16, name="r_sb",
                                         tag="r_sb", bufs=2)
                        nc.sync.dma_start(r_sb[:], rs_out[qc])
                        f_sb = wk_p.tile([P, C], F32, name="f_sb",
                                         tag="f_sb", bufs=2)
                        nc.vector.tensor_add(f_sb[:], r_sb[:], bo_bc[:])
                        nc.sync.dma_start(out[qc], f_sb[:])

                    jobs.append(rs_job)
                    return jobs

                pending = []

                def drain():
                    if pending:
                        pending.pop(0)()

                for qc in range(NQ):
                    # flash attention: head pairs interleaved at the
                    # key-block level so PE stays busy while ACT runs exp
                    nkb = 4 * qc + 4
                    for hp in range(HL // 2):
                        heads = (2 * hp, 2 * hp + 1)
                        pvs = {}
                        for h in heads:
                            pvs[h] = ps_pv.tile([P, QB], F32,
                                                name="pv_ps", tag="pv")
                        # full (unmasked) key-block pairs
                        for kp in range(2 * qc):
                            for h in heads:
                                po = 64 * (h % 2)
                                mi = h // 2
                                sT = ps_sT.tile([P, 2 * QB], F32,
                                                name="sT_ps", tag="sT")
                                for half in range(2):
                                    kb = 2 * kp + half
                                    nc.tensor.matmul(
                                        sT[:, half * QB:(half + 1) * QB],
                                        kT_sb[po:po + D, mi,
                                              kb * P:(kb + 1) * P],
                                        qT_sb[po:po + D, mi,
                                              qc * QB:(qc + 1) * QB],
                                        start=True, stop=True)
                                pT = wk_p.tile([P, 2 * QB], F32R, name="pT",
                                               tag="pT")
                                nc.scalar.activation(
                                    pT[:], sT[:],
                                    mybir.ActivationFunctionType.Exp,
                                    scale=SCALE)
                                for half in range(2):
                                    kb = 2 * kp + half
                                    nc.tensor.matmul(
                                        pvs[h][0:D + 1, :],
                                        v_sb[:, kb,
                                             h * (D + 1):(h + 1) * (D + 1)],
                                        pT[:, half * QB:(half + 1) * QB],
                                        start=(kb == 0), stop=False)
                                drain()
                        # diagonal blocks, queries narrowed to the visible
                        # range [128*di, 512); only a [128,128] tril masked
                        for di in range(4):
                            kb = 4 * qc + di
                            q0 = di * P          # first visible query col
                            qw = QB - q0
                            for h in heads:
                                po = 64 * (h % 2)
                                mi = h // 2
                                sT = ps_sT.tile([P, 2 * QB], F32,
                                                name="sT_ps", tag="sT")
                                nc.tensor.matmul(
                                    sT[:, 0:qw],
                                    kT_sb[po:po + D, mi,
                                          kb * P:(kb + 1) * P],
                                    qT_sb[po:po + D, mi,
                                          qc * QB + q0:(qc + 1) * QB],
                                    start=True, stop=True)
                                nc.vector.tensor_add(
                                    sT[:, 0:P], sT[:, 0:P], masks[:])
                                pT = wk_p.tile([P, 2 * QB], F32R, name="pT",
                                               tag="pT")
                                nc.scalar.activation(
                                    pT[:, 0:qw], sT[:, 0:qw],
                                    mybir.ActivationFunctionType.Exp,
                                    scale=SCALE)
                                nc.tensor.matmul(
                                    pvs[h][0:D + 1, q0:QB],
                                    v_sb[:, kb,
                                         h * (D + 1):(h + 1) * (D + 1)],
                                    pT[:, 0:qw],
                                    start=(qc == 0 and di == 0),
                                    stop=(di == 3))
                                drain()
                        for h in heads:
                            po = 64 * (h % 2)
                            mi = h // 2
                            rbc = wk_p.tile([D, QB], F32, name="rbc",
                                            tag="rbc", bufs=2)
                            lrow = wk_p.tile([1, QB], F32, name="lrow",
                                             tag="lrow", bufs=2)
                            nc.scalar.copy(lrow[:], pvs[h][D:D + 1, :])
                            nc.vector.reciprocal_approx_fast(
                                out=rbc[0:1, :], in_=lrow[:])
                            nc.gpsimd.partition_broadcast(rbc[:], rbc[0:1, :])
                            nc.vector.tensor_mul(
                                aoT_sb[po:po + D, mi, qc * QB:(qc + 1) * QB],
                                pvs[h][0:D, :], rbc[:])

                    # queue this chunk's output projection + RS; they are
                    # emitted inside the next chunk's attention stream
                    assert not pending
                    pending = outproj_jobs(qc)

                # drain the last chunk's jobs
                while pending:
                    drain()

    nc.compile()
    return nc


def prepare(x, w_qkv, b_qkv, w_out, b_out):
    x = np.ascontiguousarray(np.asarray(x, dtype=np.float32))
    w_qkv = np.asarray(w_qkv, dtype=np.float32)
    b_qkv = np.asarray(b_qkv, dtype=np.float32)
    w_out = np.ascontiguousarray(np.asarray(w_out, dtype=np.float32))
    b_out = np.asarray(b_out, dtype=np.float32)

    if "nc" not in _CACHED:
        _CACHED["nc"] = _build()
    nc = _CACHED["nc"]

    xTs = [np.ascontiguousarray(x[b_].T) for b_ in range(B)]
    bo = np.ascontiguousarray(b_out[None, :])
    in_maps = []
    for c in range(8):
        b_, g = c // 4, c % 4
        sl = slice(g * DL, (g + 1) * DL)
        in_maps.append({
            "xT": xTs[b_],
            "wq": np.ascontiguousarray(w_qkv[:, 0 * C:1 * C][:, sl]),
            "wk": np.ascontiguousarray(w_qkv[:, 1 * C:2 * C][:, sl]),
            "wv": np.ascontiguousarray(w_qkv[:, 2 * C:3 * C][:, sl]),
            "bq": np.ascontiguousarray(b_qkv[0 * C:1 * C][sl][None, :]),
            "bk": np.ascontiguousarray(b_qkv[1 * C:2 * C][sl][None, :]),
            "bv": np.ascontiguousarray(b_qkv[2 * C:3 * C][sl][None, :]),
            "wo": np.ascontiguousarray(w_out[g * DL:(g + 1) * DL, :]),
            "bo": bo,
        })
    return nc, in_maps


def kernel(x, w_qkv, b_qkv, w_out, b_out):
    nc, in_maps = prepare(x, w_qkv, b_qkv, w_out, b_out)
    res = run_bass_kernel_spmd(nc, in_maps, list(range(8)))
    out_full = np.empty((B, T, C), dtype=np.float32)
    for c in range(8):
        b_, g = c // 4, c % 4
        o = res.results[c]["out"]          # [NQ, P, C]
        for qc in range(NQ):
            r0 = qc * QB + g * P
            out_full[b_, r0:r0 + P, :] = o[qc]
    return out_full



# revision 14
# speedup vs baseline: 1.2540x; 1.2540x over previous
"""Distributed causal self-attention for 8 Trainium2 NeuronCores.

Problem: x[2,2048,1024] @ w_qkv[1024,3072] -> causal MHA (16 heads, d=64)
         -> @ w_out[1024,1024]. All fp32 I/O.

Sharding: core c (0..7) handles batch b=c//4 and head group g=c%4 (4 heads).
Each core projects qkv for its heads and runs flash attention. Per 512-token
query chunk, the (normalized, bf16) attention outputs are AllGather'd within
each 4-core batch group; every core then projects its own 256-column slice
of w_out locally for all 512 tokens (column-split out projection, so the
whole program stays compile-time static). Core c writes
out[b, qc*512:(qc+1)*512, 256*g:256*(g+1)].

All matmuls run in bf16 (fp32 PSUM accumulate); softmax in fp32.
Projection and attention are fused per token chunk so the PE stream starts
as soon as the weights + first x chunk land.
"""

import sys

for _p in ("/opt/trn_rl_repo", "/root/.axon_site/_ro/trn_rl_repo"):
    if _p not in sys.path:
        sys.path.insert(0, _p)

import numpy as np
import ml_dtypes

import concourse.bass as bass  # noqa: F401
import concourse.mybir as mybir
import concourse.tile as tile
from concourse import bacc
from concourse.bass_utils import run_bass_kernel_spmd

P = 128
B, T, C = 2, 2048, 1024
H, D = 16, 64
HL = 4               # heads per core
DL = HL * D          # 256 local head dims
KC = C // P          # 8 contraction tiles over C
QB = 512             # query chunk
NQ = T // QB         # 4 query chunks
NT = T // P          # 16 token tiles
G = 4                # cores per batch group
CW = C // G          # 256 output columns per core
SCALE = 1.0 / 8.0    # 1/sqrt(64)
NEG = -1.0e30

F32 = mybir.dt.float32
BF16 = mybir.dt.bfloat16
NPBF16 = ml_dtypes.bfloat16

_CACHED = {}


def _mask_data():
    # tril mask: 0 where key j <= query i, NEG above the diagonal
    j = np.arange(P)[:, None]
    i = np.arange(P)[None, :]
    return np.where(j <= i, 0.0, NEG).astype(np.float32)


def _build():
    nc = bacc.Bacc("TRN2", target_bir_lowering=False, debug=False,
                   num_devices=8)

    xT = nc.dram_tensor("xT", [C, T], BF16, kind="ExternalInput")
    wq = nc.dram_tensor("wq", [C, DL], BF16, kind="ExternalInput")
    wk = nc.dram_tensor("wk", [C, DL], BF16, kind="ExternalInput")
    wv = nc.dram_tensor("wv", [C, DL], BF16, kind="ExternalInput")
    bq = nc.dram_tensor("bq", [1, DL], F32, kind="ExternalInput")
    bk = nc.dram_tensor("bk", [1, DL], F32, kind="ExternalInput")
    bv = nc.dram_tensor("bv", [1, DL], BF16, kind="ExternalInput")
    wo = nc.dram_tensor("wo", [C, CW], BF16, kind="ExternalInput")
    bo = nc.dram_tensor("bo", [1, CW], F32, kind="ExternalInput")
    # per query-chunk column slice: rows qc*512.., cols g*256..
    out = nc.dram_tensor("out", [NQ, QB, CW], F32, kind="ExternalOutput")

    masks_dram = nc.inline_tensor(_mask_data(), name="cmasks")
    GROUPS = [[0, 1, 2, 3], [4, 5, 6, 7]]

    with tile.TileContext(nc) as tc:
        with (
            tc.tile_pool(name="const", bufs=1) as cp,
            tc.tile_pool(name="persist", bufs=1) as pp,
            tc.tile_pool(name="work", bufs=3) as wk_p,
            tc.tile_pool(name="dram", bufs=1, space="DRAM") as dp,
            tc.tile_pool(name="ps_a", bufs=2, space="PSUM") as ps_a,
            tc.tile_pool(name="ps_sT", bufs=2, space="PSUM") as ps_sT,
            tc.tile_pool(name="ps_pv", bufs=2, space="PSUM") as ps_pv,
        ):
            # ---- constants (small, load first) ----
            masks = cp.tile([P, P], F32)
            nc.sync.dma_start(masks[:], masks_dram[:])
            ones_f = cp.tile([1, QB], F32)
            nc.vector.memset(ones_f[:], 1.0)
            ones_b = cp.tile([1, QB], BF16)
            nc.vector.tensor_copy(ones_b[:], ones_f[:])
            # q/k biases as per-partition columns [128, 2] (mi-major)
            bq_col = cp.tile([P, 2], F32)
            bk_col = cp.tile([P, 2], F32)
            nc.sync.dma_start(
                bq_col[:], bq[0, :].rearrange("(m p) -> p m", p=P))
            nc.sync.dma_start(
                bk_col[:], bk[0, :].rearrange("(m p) -> p m", p=P))
            bv_sb = cp.tile([1, DL], BF16)
            nc.sync.dma_start(bv_sb[:], bv[:])
            bo_sb = cp.tile([1, CW], F32)
            nc.sync.dma_start(bo_sb[:], bo[:])
            bo_bc = cp.tile([P, CW], F32)
            nc.gpsimd.partition_broadcast(bo_bc[:], bo_sb[:])

            # ---- weights (qkv first: needed for chunk-0 projection) ----
            wq_sb = cp.tile([P, KC, DL], BF16)
            wk_sb = cp.tile([P, KC, DL], BF16)
            wv_sb = cp.tile([P, KC, DL], BF16)
            nc.sync.dma_start(wq_sb[:], wq.rearrange("(k p) m -> p k m", p=P))
            nc.sync.dma_start(wk_sb[:], wk.rearrange("(k p) m -> p k m", p=P))
            nc.sync.dma_start(wv_sb[:], wv.rearrange("(k p) m -> p k m", p=P))

            # ---- persistent activations (bf16) ----
            qT_sb = pp.tile([P, 2, T], BF16)     # [d, t], d = mi*128+p
            kT_sb = pp.tile([P, 2, T], BF16)
            v_sb = pp.tile([P, NT, HL * (D + 1)], BF16)  # per head: 64 v + 1
            # ones columns of v_sb (softmax denominator accumulator)
            ones64 = cp.tile([P, NT * HL], BF16)
            nc.vector.memset(ones64[:], 1.0)
            vones = v_sb.rearrange("p n (h e) -> p n h e", h=HL)[:, :, :, D:D + 1]
            nc.vector.tensor_copy(vones, ones64[:].rearrange(
                "p (n h) -> p n h", n=NT)[:, :, :, None])

            # out-proj weights (after qkv weights; needed from chunk 1 on)
            wo_sb = cp.tile([P, KC, CW], BF16)
            nc.sync.dma_start(wo_sb[:], wo.rearrange("(k p) n -> p k n", p=P))

            # AllGather staging (internal DRAM); the last chunk runs one AG
            # per head-pair slab (mi-major output) so the tail only waits on
            # the second slab
            ag_in = dp.tile([NQ, 2, P, QB], BF16)
            ag_out = dp.tile([NQ - 1, G, 2, P, QB], BF16)
            ag_out_last = dp.tile([2, G, P, QB], BF16)

            xT_r = xT.rearrange("(k p) t -> p k t", p=P)

            pending = []

            def drain():
                if pending:
                    pending.pop(0)()

            def outproj_jobs(qc):
                # gather + 4 column-split out-proj groups + writeback for
                # chunk qc; drained inside the NEXT chunk's attention stream.
                jobs = []

                def gath_job():
                    gath = wk_p.tile([P, KC, QB], BF16, name="gath",
                                     tag="gath", bufs=2)
                    gv = gath[:].rearrange("p (j mi) t -> p j mi t",
                                           j=4, mi=2)
                    for mi in range(2):
                        if qc == NQ - 1:
                            src = ag_out_last[mi]
                        else:
                            src = ag_out[qc][:, mi]
                        nc.sync.dma_start(
                            gv[:, :, mi, :],
                            src.rearrange("j p t -> p j t"))
                    return gath

                holder = {}

                def j_gath():
                    holder["g"] = gath_job()

                jobs.append(j_gath)

                f_sb = {}

                def tt_job(ti):
                    gath = holder["g"]
                    ps = ps_a.tile([P, QB], F32, name="ps_a", tag="ps_a")
                    for kk in range(KC):
                        nc.tensor.matmul(
                            ps[:, 0:CW],
                            gath[:, kk, ti * P:(ti + 1) * P],
                            wo_sb[:, kk, :],
                            start=(kk == 0), stop=(kk == KC - 1))
                    if ti == 0:
                        f_sb["t"] = wk_p.tile([P, 4, CW], F32, name="f_sb",
                                              tag="f_sb", bufs=2)
                    nc.vector.tensor_add(f_sb["t"][:, ti, :], ps[:, 0:CW],
                                         bo_bc[:])

                for ti in range(4):
                    jobs.append(lambda ti=ti: tt_job(ti))

                def wb_job():
                    nc.sync.dma_start(
                        out[qc].rearrange("(tt p) n -> p tt n", p=P),
                        f_sb["t"][:])

                jobs.append(wb_job)
                return jobs

            def xc_fetch(tc_i):
                xc = wk_p.tile([P, KC, QB], BF16, name="xc", tag="xc", bufs=2)
                nc.sync.dma_start(
                    xc[:], xT_r[:, :, tc_i * QB:(tc_i + 1) * QB])
                return xc

            xc_next = xc_fetch(0)
            for tc_i in range(NQ):
                xc = xc_next

                # ---- qkv projection for this chunk ----
                for w_sb, b_col, dst in ((wq_sb, bq_col, qT_sb),
                                         (wk_sb, bk_col, kT_sb)):
                    for mi in range(2):
                        ps = ps_a.tile([P, QB], F32, name="ps_a", tag="ps_a")
                        for kk in range(KC):
                            nc.tensor.matmul(
                                ps[:],
                                w_sb[:, kk, mi * P:(mi + 1) * P],
                                xc[:, kk, :],
                                start=(kk == 0), stop=(kk == KC - 1))
                        nc.vector.tensor_scalar_add(
                            dst[:, mi, tc_i * QB:(tc_i + 1) * QB], ps[:],
                            b_col[:, mi:mi + 1])
                for t4 in range(4):
                    ti = tc_i * 4 + t4
                    ps = ps_a.tile([P, QB], F32, name="ps_a", tag="ps_a")
                    for kk in range(KC):
                        nc.tensor.matmul(
                            ps[:, 0:DL],
                            xc[:, kk, t4 * P:(t4 + 1) * P],
                            wv_sb[:, kk, :],
                            start=(kk == 0), stop=False)
                    nc.tensor.matmul(ps[:, 0:DL], ones_b[:, :P], bv_sb[:],
                                     start=False, stop=True)
                    nc.vector.tensor_copy(
                        v_sb.rearrange("p n (h e) -> p n h e", h=HL)
                        [:, ti, :, 0:D],
                        ps[:, 0:DL].rearrange("p (h e) -> p h e", e=D))

                # prefetch next x chunk during this chunk's attention
                if tc_i + 1 < NQ:
                    xc_next = xc_fetch(tc_i + 1)

                # ---- attention for this chunk ----
                qc = tc_i
                n_jobs = len(pending)
                n_units = 8 * qc + 8        # drainable units this chunk
                drain_start = max(0, n_units - n_jobs - 2)
                unit = 0

                def unit_drain():
                    nonlocal unit
                    if unit >= drain_start:
                        drain()
                    unit += 1

                aoT_c = wk_p.tile([P, 2, QB], BF16, name="aoT", tag="aoT",
                                  bufs=2)
                for hp in range(2):
                    mi = hp
                    heads = (2 * hp, 2 * hp + 1)
                    pvs = {}
                    for h in heads:
                        pvs[h] = ps_pv.tile([P, QB], F32,
                                            name="pv_ps", tag="pv")
                    # full (unmasked) key-block pairs
                    for kp in range(2 * qc):
                        for h in heads:
                            po = 64 * (h % 2)
                            sT = ps_sT.tile([P, 2 * QB], F32,
                                            name="sT_ps", tag="sT")
                            for half in range(2):
                                kb = 2 * kp + half
                                nc.tensor.matmul(
                                    sT[:, half * QB:(half + 1) * QB],
                                    kT_sb[po:po + D, mi,
                                          kb * P:(kb + 1) * P],
                                    qT_sb[po:po + D, mi,
                                          qc * QB:(qc + 1) * QB],
                                    start=True, stop=True)
                            pT = wk_p.tile([P, 2 * QB], BF16, name="pT",
                                           tag="pT")
                            nc.scalar.activation(
                                pT[:], sT[:],
                                mybir.ActivationFunctionType.Exp,
                                scale=SCALE)
                            for half in range(2):
                                kb = 2 * kp + half
                                nc.tensor.matmul(
                                    pvs[h][0:D + 1, :],
                                    v_sb[:, kb,
                                         h * (D + 1):(h + 1) * (D + 1)],
                                    pT[:, half * QB:(half + 1) * QB],
                                    start=(kb == 0), stop=False)
                            unit_drain()
                    # diagonal blocks: queries narrowed to [128*di, 512);
                    # both heads share one sT/pT tile (one exp inst)
                    for di in range(4):
                        kb = 4 * qc + di
                        q0 = di * P
                        qw = QB - q0
                        # head0 at [q0:512] (end of bank A), head1 at
                        # [512:512+qw] (start of bank B): each MM stays in
                        # one PSUM bank, exp region [q0:1024-q0] contiguous
                        off = (q0, QB)
                        sT = ps_sT.tile([P, 2 * QB], F32,
                                        name="sT_ps", tag="sT")
                        for idx, h in enumerate(heads):
                            po = 64 * (h % 2)
                            nc.tensor.matmul(
                                sT[:, off[idx]:off[idx] + qw],
                                kT_sb[po:po + D, mi,
                                      kb * P:(kb + 1) * P],
                                qT_sb[po:po + D, mi,
                                      qc * QB + q0:(qc + 1) * QB],
                                start=True, stop=True)
                        for idx in range(2):
                            nc.vector.tensor_add(
                                sT[:, off[idx]:off[idx] + P],
                                sT[:, off[idx]:off[idx] + P], masks[:])
                        pT = wk_p.tile([P, 2 * QB], BF16, name="pT",
                                       tag="pT")
                        nc.scalar.activation(
                            pT[:, q0:2 * QB - q0], sT[:, q0:2 * QB - q0],
                            mybir.ActivationFunctionType.Exp,
                            scale=SCALE)
                        for idx, h in enumerate(heads):
                            nc.tensor.matmul(
                                pvs[h][0:D + 1, q0:QB],
                                v_sb[:, kb,
                                     h * (D + 1):(h + 1) * (D + 1)],
                                pT[:, off[idx]:off[idx] + qw],
                                start=(qc == 0 and di == 0),
                                stop=(di == 3))
                        unit_drain()
                    # normalize both heads into aoT_c
                    for h in heads:
                        po = 64 * (h % 2)
                        den = wk_p.tile([1, QB], F32, name="den",
                                        tag="den", bufs=2)
                        nc.vector.tensor_copy(den[:], pvs[h][D:D + 1, :])
                        rbc = wk_p.tile([D, QB], F32, name="rbc",
                                        tag="rbc", bufs=2)
                        nc.vector.reciprocal_approx_fast(
                            out=rbc[0:1, :], in_=den[:])
                        nc.gpsimd.partition_broadcast(rbc[:], rbc[0:1, :])
                        nc.vector.tensor_mul(
                            aoT_c[po:po + D, mi, :],
                            pvs[h][0:D, :], rbc[:])
                    # ship this head-pair's slab; on the last chunk launch
                    # the AG per-slab so the tail only waits on half
                    nc.sync.dma_start(ag_in[qc, mi], aoT_c[:, mi, :])
                    if qc == NQ - 1:
                        nc.gpsimd.collective_compute(
                            "AllGather", mybir.AluOpType.bypass,
                            replica_groups=GROUPS,
                            ins=[ag_in[qc, mi]],
                            outs=[ag_out_last[mi]],
                        )
                if qc != NQ - 1:
                    nc.gpsimd.collective_compute(
                        "AllGather", mybir.AluOpType.bypass,
                        replica_groups=GROUPS,
                        ins=[ag_in[qc]],
                        outs=[ag_out[qc]],
                    )

                assert not pending, f"undrained jobs at chunk {qc}"
                pending = outproj_jobs(qc)

            # tail: the last chunk's gather/out-proj/writeback
            while pending:
                drain()

    nc.compile()
    return nc


def prepare(x, w_qkv, b_qkv, w_out, b_out):
    x = np.ascontiguousarray(np.asarray(x, dtype=np.float32))
    w_qkv = np.asarray(w_qkv, dtype=np.float32)
    b_qkv = np.asarray(b_qkv, dtype=np.float32)
    w_out = np.ascontiguousarray(np.asarray(w_out, dtype=np.float32))
    b_out = np.asarray(b_out, dtype=np.float32)

    if "nc" not in _CACHED:
        _CACHED["nc"] = _build()
    nc = _CACHED["nc"]

    xTs = [np.ascontiguousarray(x[b_].T.astype(NPBF16)) for b_ in range(B)]
    in_maps = []
    for c in range(8):
        b_, g = c // 4, c % 4
        sl = slice(g * DL, (g + 1) * DL)
        cs = slice(g * CW, (g + 1) * CW)
        in_maps.append({
            "xT": xTs[b_],
            "wq": np.ascontiguousarray(
                w_qkv[:, 0 * C:1 * C][:, sl].astype(NPBF16)),
            "wk": np.ascontiguousarray(
                w_qkv[:, 1 * C:2 * C][:, sl].astype(NPBF16)),
            "wv": np.ascontiguousarray(
                w_qkv[:, 2 * C:3 * C][:, sl].astype(NPBF16)),
            "bq": np.ascontiguousarray(b_qkv[0 * C:1 * C][sl][None, :]),
            "bk": np.ascontiguousarray(b_qkv[1 * C:2 * C][sl][None, :]),
            "bv": np.ascontiguousarray(
                b_qkv[2 * C:3 * C][sl][None, :].astype(NPBF16)),
            "wo": np.ascontiguousarray(w_out[:, cs].astype(NPBF16)),
            "bo": np.ascontiguousarray(b_out[cs][None, :]),
        })
    return nc, in_maps


def kernel(x, w_qkv, b_qkv, w_out, b_out):
    nc, in_maps = prepare(x, w_qkv, b_qkv, w_out, b_out)
    res = run_bass_kernel_spmd(nc, in_maps, list(range(8)))
    out_full = np.empty((B, T, C), dtype=np.float32)
    for c in range(8):
        b_, g = c // 4, c % 4
        o = res.results[c]["out"]          # [NQ, QB, CW]
        for qc in range(NQ):
            out_full[b_, qc * QB:(qc + 1) * QB,
                     g * CW:(g + 1) * CW] = o[qc]
    return out_full


# revision 22
# speedup vs baseline: 1.5298x; 1.2199x over previous
"""Distributed causal self-attention for 8 Trainium2 NeuronCores.

Problem: x[2,2048,1024] @ w_qkv[1024,3072] -> causal MHA (16 heads, d=64)
         -> @ w_out[1024,1024]. All fp32 I/O.

Sharding: core c (0..7) handles batch b=c//4 and head group g=c%4 (4 heads).
Each core projects qkv for its heads and runs flash attention. Per 512-token
query chunk, the (normalized, bf16) attention outputs are AllGather'd within
each 4-core batch group; every core then projects its own 256-column slice
of w_out locally for all 512 tokens (column-split out projection, so the
whole program stays compile-time static). Core c writes
out[b, qc*512:(qc+1)*512, 256*g:256*(g+1)].

All matmuls run in bf16 (fp32 PSUM accumulate); softmax in fp32.
Projection and attention are fused per token chunk so the PE stream starts
as soon as the weights + first x chunk land.
"""

import sys

for _p in ("/opt/trn_rl_repo", "/root/.axon_site/_ro/trn_rl_repo"):
    if _p not in sys.path:
        sys.path.insert(0, _p)

import numpy as np
import ml_dtypes

import concourse.bass as bass  # noqa: F401
import concourse.mybir as mybir
import concourse.tile as tile
from concourse import bacc
from concourse.bass_utils import run_bass_kernel_spmd

P = 128
B, T, C = 2, 2048, 1024
H, D = 16, 64
HL = 4               # heads per core
DL = HL * D          # 256 local head dims
KC = C // P          # 8 contraction tiles over C
QB = 512             # query chunk
NQ = T // QB         # 4 query chunks
NT = T // P          # 16 token tiles
G = 4                # cores per batch group
CW = C // G          # 256 output columns per core
SCALE = 1.0 / 8.0    # 1/sqrt(64)
NEG = -1.0e30

F32 = mybir.dt.float32
BF16 = mybir.dt.bfloat16
NPBF16 = ml_dtypes.bfloat16

_CACHED = {}


def _mask_data():
    # tril mask: 0 where key j <= query i, NEG above the diagonal
    j = np.arange(P)[:, None]
    i = np.arange(P)[None, :]
    return np.where(j <= i, 0.0, NEG).astype(np.float32)


def _build():
    nc = bacc.Bacc("TRN2", target_bir_lowering=False, debug=False,
                   num_devices=8)

    xT = nc.dram_tensor("xT", [C, T], BF16, kind="ExternalInput")
    wq = nc.dram_tensor("wq", [C, DL], BF16, kind="ExternalInput")
    wk = nc.dram_tensor("wk", [C, DL], BF16, kind="ExternalInput")
    wv = nc.dram_tensor("wv", [C, DL], BF16, kind="ExternalInput")
    bq = nc.dram_tensor("bq", [1, DL], F32, kind="ExternalInput")
    bk = nc.dram_tensor("bk", [1, DL], F32, kind="ExternalInput")
    bv = nc.dram_tensor("bv", [1, DL], BF16, kind="ExternalInput")
    wo = nc.dram_tensor("wo", [C, CW], BF16, kind="ExternalInput")
    bo = nc.dram_tensor("bo", [1, CW], F32, kind="ExternalInput")
    # per query-chunk column slice: rows qc*512.., cols g*256..
    out = nc.dram_tensor("out", [NQ, QB, CW], F32, kind="ExternalOutput")

    masks_dram = nc.inline_tensor(_mask_data(), name="cmasks")
    GROUPS = [[0, 1, 2, 3], [4, 5, 6, 7]]

    with tile.TileContext(nc) as tc:
        with (
            tc.tile_pool(name="const", bufs=1) as cp,
            tc.tile_pool(name="persist", bufs=1) as pp,
            tc.tile_pool(name="work", bufs=3) as wk_p,
            tc.tile_pool(name="dram", bufs=1, space="DRAM") as dp,
            tc.tile_pool(name="ps_a", bufs=2, space="PSUM") as ps_a,
            tc.tile_pool(name="ps_sT", bufs=2, space="PSUM") as ps_sT,
            tc.tile_pool(name="ps_pv", bufs=2, space="PSUM") as ps_pv,
        ):
            # ---- constants (small, load first) ----
            masks = cp.tile([P, P], F32)
            nc.sync.dma_start(masks[:], masks_dram[:])
            ones_f = cp.tile([1, QB], F32)
            nc.vector.memset(ones_f[:], 1.0)
            ones_b = cp.tile([1, QB], BF16)
            nc.vector.tensor_copy(ones_b[:], ones_f[:])
            # q/k biases as per-partition columns [128, 2] (mi-major)
            bq_col = cp.tile([P, 2], F32)
            bk_col = cp.tile([P, 2], F32)
            nc.sync.dma_start(
                bq_col[:], bq[0, :].rearrange("(m p) -> p m", p=P))
            nc.sync.dma_start(
                bk_col[:], bk[0, :].rearrange("(m p) -> p m", p=P))
            bv_sb = cp.tile([1, DL], BF16)
            nc.sync.dma_start(bv_sb[:], bv[:])
            bo_sb = cp.tile([1, CW], F32)
            nc.sync.dma_start(bo_sb[:], bo[:])
            bo_bc = cp.tile([P, CW], F32)
            nc.gpsimd.partition_broadcast(bo_bc[:], bo_sb[:])

            # ---- weights; wq first so the chunk-0 q-projection can start
            # as soon as wq + x0 land ----
            wq_sb = cp.tile([P, KC, DL], BF16)
            wk_sb = cp.tile([P, KC, DL], BF16)
            wv_sb = cp.tile([P, KC, DL], BF16)
            nc.sync.dma_start(wq_sb[:], wq.rearrange("(k p) m -> p k m", p=P))

            # ---- persistent activations (bf16) ----
            qT_sb = pp.tile([P, 2, T], BF16)     # [d, t], d = mi*128+p
            kT_sb = pp.tile([P, 2, T], BF16)
            v_sb = pp.tile([P, NT, HL * (D + 1)], BF16)  # per head: 64 v + 1
            # ones columns of v_sb (softmax denominator accumulator)
            ones64 = cp.tile([P, NT * HL], BF16)
            nc.vector.memset(ones64[:], 1.0)
            vones = v_sb.rearrange("p n (h e) -> p n h e", h=HL)[:, :, :, D:D + 1]
            nc.vector.tensor_copy(vones, ones64[:].rearrange(
                "p (n h) -> p n h", n=NT)[:, :, :, None])

            # out-proj weights (DMA emitted after wk/wv; first needed in
            # chunk 3's attention)
            wo_sb = cp.tile([P, KC, CW], BF16)

            # AllGather staging (internal DRAM); the last chunk runs one AG
            # per head-pair slab (mi-major output) so the tail only waits on
            # the second slab
            ag_in = dp.tile([NQ, 2, P, QB], BF16)
            ag_out = dp.tile([NQ - 1, G, 2, P, QB], BF16)
            ag_out_last = dp.tile([2, G, P, QB], BF16)

            xT_r = xT.rearrange("(k p) t -> p k t", p=P)

            # deferred out-proj jobs: the collectives channel needs a ~45us
            # bootstrap barrier before the first AG completes, so chunks
            # 0-2's jobs all drain inside chunk 3's attention (their AGs are
            # done by then); chunk 3's jobs form the tail
            pending = []

            def drain():
                if pending:
                    pending.pop(0)()

            def outproj_jobs(qc):
                # gather + 4 column-split out-proj groups + writeback for
                # chunk qc; drained inside the NEXT chunk's attention stream.
                jobs = []

                def gath_job():
                    gath = wk_p.tile([P, KC, QB], BF16, name="gath",
                                     tag="gath", bufs=2)
                    gv = gath[:].rearrange("p (j mi) t -> p j mi t",
                                           j=4, mi=2)
                    for mi in range(2):
                        if qc == NQ - 1:
                            src = ag_out_last[mi]
                        else:
                            src = ag_out[qc][:, mi]
                        nc.sync.dma_start(
                            gv[:, :, mi, :],
                            src.rearrange("j p t -> p j t"))
                    return gath

                holder = {}

                def j_gath():
                    holder["g"] = gath_job()

                jobs.append(j_gath)

                f_sb = {}

                def tt_job(ti):
                    gath = holder["g"]
                    ps = ps_a.tile([P, QB], F32, name="ps_a", tag="ps_a")
                    for kk in range(KC):
                        nc.tensor.matmul(
                            ps[:, 0:CW],
                            gath[:, kk, ti * P:(ti + 1) * P],
                            wo_sb[:, kk, :],
                            start=(kk == 0), stop=(kk == KC - 1))
                    if ti == 0:
                        f_sb["t"] = wk_p.tile([P, 4, CW], F32, name="f_sb",
                                              tag="f_sb", bufs=2)
                    nc.vector.tensor_add(f_sb["t"][:, ti, :], ps[:, 0:CW],
                                         bo_bc[:])

                for ti in range(4):
                    jobs.append(lambda ti=ti: tt_job(ti))

                def wb_job():
                    nc.sync.dma_start(
                        out[qc].rearrange("(tt p) n -> p tt n", p=P),
                        f_sb["t"][:])

                jobs.append(wb_job)
                return jobs

            def xc_fetch(tc_i):
                xc = wk_p.tile([P, KC, QB], BF16, name="xc", tag="xc", bufs=2)
                nc.sync.dma_start(
                    xc[:], xT_r[:, :, tc_i * QB:(tc_i + 1) * QB])
                return xc

            xc_next = xc_fetch(0)
            nc.sync.dma_start(wk_sb[:], wk.rearrange("(k p) m -> p k m", p=P))
            nc.sync.dma_start(wv_sb[:], wv.rearrange("(k p) m -> p k m", p=P))
            nc.sync.dma_start(wo_sb[:], wo.rearrange("(k p) n -> p k n", p=P))
            for tc_i in range(NQ):
                xc = xc_next

                # ---- qkv projection for this chunk ----
                for w_sb, b_col, dst in ((wq_sb, bq_col, qT_sb),
                                         (wk_sb, bk_col, kT_sb)):
                    for mi in range(2):
                        ps = ps_a.tile([P, QB], F32, name="ps_a", tag="ps_a")
                        for kk in range(KC):
                            nc.tensor.matmul(
                                ps[:],
                                w_sb[:, kk, mi * P:(mi + 1) * P],
                                xc[:, kk, :],
                                start=(kk == 0), stop=(kk == KC - 1))
                        nc.vector.tensor_scalar_add(
                            dst[:, mi, tc_i * QB:(tc_i + 1) * QB], ps[:],
                            b_col[:, mi:mi + 1])
                for t4 in range(4):
                    ti = tc_i * 4 + t4
                    ps = ps_a.tile([P, QB], F32, name="ps_a", tag="ps_a")
                    for kk in range(KC):
                        nc.tensor.matmul(
                            ps[:, 0:DL],
                            xc[:, kk, t4 * P:(t4 + 1) * P],
                            wv_sb[:, kk, :],
                            start=(kk == 0), stop=False)
                    nc.tensor.matmul(ps[:, 0:DL], ones_b[:, :P], bv_sb[:],
                                     start=False, stop=True)
                    nc.vector.tensor_copy(
                        v_sb.rearrange("p n (h e) -> p n h e", h=HL)
                        [:, ti, :, 0:D],
                        ps[:, 0:DL].rearrange("p (h e) -> p h e", e=D))

                # prefetch next x chunk during this chunk's attention
                if tc_i + 1 < NQ:
                    xc_next = xc_fetch(tc_i + 1)

                # ---- attention for this chunk ----
                qc = tc_i
                n_jobs = len(pending)
                n_units = 8 * qc + 8        # drainable units this chunk
                if qc < NQ - 1:
                    drain_start = n_units   # defer everything to chunk 3
                else:
                    drain_start = max(0, n_units - n_jobs - 4)
                unit = 0

                def unit_drain():
                    nonlocal unit
                    if unit >= drain_start:
                        drain()
                    unit += 1

                aoT_c = wk_p.tile([P, 2, QB], BF16, name="aoT", tag="aoT",
                                  bufs=2)
                for hp in range(2):
                    mi = hp
                    heads = (2 * hp, 2 * hp + 1)
                    pvs = {}
                    for h in heads:
                        pvs[h] = ps_pv.tile([P, QB], F32,
                                            name="pv_ps", tag="pv")
                    # full (unmasked) key-block pairs; both heads' score MMs
                    # emitted back-to-back so they run concurrently on PE
                    # row-groups (0,*) and (64,*) (K=64 each)
                    for kp in range(2 * qc):
                        sTs, pTs = {}, {}
                        for h in heads:
                            po = 64 * (h % 2)
                            sTs[h] = ps_sT.tile([P, 2 * QB], F32,
                                                name="sT_ps", tag="sT")
                            for half in range(2):
                                kb = 2 * kp + half
                                nc.tensor.matmul(
                                    sTs[h][:, half * QB:(half + 1) * QB],
                                    kT_sb[po:po + D, mi,
                                          kb * P:(kb + 1) * P],
                                    qT_sb[po:po + D, mi,
                                          qc * QB:(qc + 1) * QB],
                                    start=True, stop=True)
                        for h in heads:
                            pTs[h] = wk_p.tile([P, 2 * QB], BF16, name="pT",
                                               tag="pT")
                            nc.scalar.activation(
                                pTs[h][:], sTs[h][:],
                                mybir.ActivationFunctionType.Exp,
                                scale=SCALE)
                        for h in heads:
                            for half in range(2):
                                kb = 2 * kp + half
                                nc.tensor.matmul(
                                    pvs[h][0:D + 1, :],
                                    v_sb[:, kb,
                                         h * (D + 1):(h + 1) * (D + 1)],
                                    pTs[h][:, half * QB:(half + 1) * QB],
                                    start=(kb == 0), stop=False)
                            unit_drain()
                    # diagonal blocks: queries narrowed to [128*di, 512);
                    # both heads share one sT/pT tile (one exp inst)
                    for di in range(4):
                        kb = 4 * qc + di
                        q0 = di * P
                        qw = QB - q0
                        # head0 at [q0:512] (end of bank A), head1 at
                        # [512:512+qw] (start of bank B): each MM stays in
                        # one PSUM bank, exp region [q0:1024-q0] contiguous
                        off = (q0, QB)
                        sT = ps_sT.tile([P, 2 * QB], F32,
                                        name="sT_ps", tag="sT")
                        for idx, h in enumerate(heads):
                            po = 64 * (h % 2)
                            nc.tensor.matmul(
                                sT[:, off[idx]:off[idx] + qw],
                                kT_sb[po:po + D, mi,
                                      kb * P:(kb + 1) * P],
                                qT_sb[po:po + D, mi,
                                      qc * QB + q0:(qc + 1) * QB],
                                start=True, stop=True)
                        for idx in range(2):
                            nc.vector.tensor_add(
                                sT[:, off[idx]:off[idx] + P],
                                sT[:, off[idx]:off[idx] + P], masks[:])
                        pT = wk_p.tile([P, 2 * QB], BF16, name="pT",
                                       tag="pT")
                        nc.scalar.activation(
                            pT[:, q0:2 * QB - q0], sT[:, q0:2 * QB - q0],
                            mybir.ActivationFunctionType.Exp,
                            scale=SCALE)
                        for idx, h in enumerate(heads):
                            nc.tensor.matmul(
                                pvs[h][0:D + 1, q0:QB],
                                v_sb[:, kb,
                                     h * (D + 1):(h + 1) * (D + 1)],
                                pT[:, off[idx]:off[idx] + qw],
                                start=(qc == 0 and di == 0),
                                stop=(di == 3))
                        unit_drain()
                    # normalize both heads into aoT_c
                    for h in heads:
                        po = 64 * (h % 2)
                        den = wk_p.tile([1, QB], F32, name="den",
                                        tag="den", bufs=2)
                        nc.vector.tensor_copy(den[:], pvs[h][D:D + 1, :])
                        rbc = wk_p.tile([D, QB], F32, name="rbc",
                                        tag="rbc", bufs=2)
                        nc.vector.reciprocal_approx_fast(
                            out=rbc[0:1, :], in_=den[:])
                        nc.gpsimd.partition_broadcast(rbc[:], rbc[0:1, :])
                        nc.vector.tensor_mul(
                            aoT_c[po:po + D, mi, :],
                            pvs[h][0:D, :], rbc[:])
                    # ship this head-pair's slab; on the last chunk launch
                    # the AG per-slab so the tail only waits on half
                    nc.sync.dma_start(ag_in[qc, mi], aoT_c[:, mi, :])
                    if qc == NQ - 1:
                        nc.gpsimd.collective_compute(
                            "AllGather", mybir.AluOpType.bypass,
                            replica_groups=GROUPS,
                            ins=[ag_in[qc, mi]],
                            outs=[ag_out_last[mi]],
                        )
                if qc != NQ - 1:
                    nc.gpsimd.collective_compute(
                        "AllGather", mybir.AluOpType.bypass,
                        replica_groups=GROUPS,
                        ins=[ag_in[qc]],
                        outs=[ag_out[qc]],
                    )

                pending.extend(outproj_jobs(qc))

            # tail: the last chunk's gather/out-proj/writeback
            while pending:
                drain()

    nc.compile()
    return nc


def prepare(x, w_qkv, b_qkv, w_out, b_out):
    x = np.ascontiguousarray(np.asarray(x, dtype=np.float32))
    w_qkv = np.asarray(w_qkv, dtype=np.float32)
    b_qkv = np.asarray(b_qkv, dtype=np.float32)
    w_out = np.ascontiguousarray(np.asarray(w_out, dtype=np.float32))
    b_out = np.asarray(b_out, dtype=np.float32)

    if "nc" not in _CACHED:
        _CACHED["nc"] = _build()
    nc = _CACHED["nc"]

    xTs = [np.ascontiguousarray(x[b_].T.astype(NPBF16)) for b_ in range(B)]
    in_maps = []
    for c in range(8):
        b_, g = c // 4, c % 4
        sl = slice(g * DL, (g + 1) * DL)
        cs = slice(g * CW, (g + 1) * CW)
        in_maps.append({
            "xT": xTs[b_],
            "wq": np.ascontiguousarray(
                w_qkv[:, 0 * C:1 * C][:, sl].astype(NPBF16)),
            "wk": np.ascontiguousarray(
                w_qkv[:, 1 * C:2 * C][:, sl].astype(NPBF16)),
            "wv": np.ascontiguousarray(
                w_qkv[:, 2 * C:3 * C][:, sl].astype(NPBF16)),
            "bq": np.ascontiguousarray(b_qkv[0 * C:1 * C][sl][None, :]),
            "bk": np.ascontiguousarray(b_qkv[1 * C:2 * C][sl][None, :]),
            "bv": np.ascontiguousarray(
                b_qkv[2 * C:3 * C][sl][None, :].astype(NPBF16)),
            "wo": np.ascontiguousarray(w_out[:, cs].astype(NPBF16)),
            "bo": np.ascontiguousarray(b_out[cs][None, :]),
        })
    return nc, in_maps


def kernel(x, w_qkv, b_qkv, w_out, b_out):
    nc, in_maps = prepare(x, w_qkv, b_qkv, w_out, b_out)
    res = run_bass_kernel_spmd(nc, in_maps, list(range(8)))
    out_full = np.empty((B, T, C), dtype=np.float32)
    for c in range(8):
        b_, g = c // 4, c % 4
        o = res.results[c]["out"]          # [NQ, QB, CW]
        for qc in range(NQ):
            out_full[b_, qc * QB:(qc + 1) * QB,
                     g * CW:(g + 1) * CW] = o[qc]
    return out_full
